# revision 2
# baseline (speedup 1.0000x reference)
"""Trainium2 Bass kernel for causal multi-head attention (8-core SPMD).

Problem: B=2, S=2048, H=2048, 16 heads (hd=128), RoPE, causal mask,
layer-index scaling (/4), additive pad mask (zeros by construction).

Sharding: core c handles batch b=c//4 and head-group g=c%4 (4 heads).
wq/wk/wv column-parallel, wo row-parallel; host sums the 4 partial
outputs per batch.

Per-core dataflow (all feature-on-partition, "transposed" layouts):
  qT/kT [d=128, S] = w.T-tile @ xT      (PSUM accum over 16 H-chunks)
  RoPE on qT/kT via head-dim permutation chosen so the rotation pair
    sits 16 partitions apart inside each 32-partition quadrant
    (stream_shuffle does the swap in one DVE op)
  scoresT [k,q] tiles = kT-tile.T @ qT-block ; exp on ACT -> PT (bf16)
  diagonal tiles masked multiplicatively post-exp
  row sums via all-ones stationary matmul (broadcast over partitions)
  OT [d, q] += v-tile.T @ PT ; normalized by reciprocal(sums)
  out_partial = OT.T @ woT  (PSUM accum over the 4 local head chunks)

Matmuls run in bf16 (fp32 PSUM accumulation); softmax math in fp32.
"""
import math
import os
import sys

import numpy as np

for _p in ("/opt/trn_rl_repo", "/root/.axon_site/_ro/trn_rl_repo"):
    if os.path.isdir(_p) and _p not in sys.path:
        sys.path.append(_p)

import ml_dtypes

S = 2048
H = 2048
NHEADS = 16
HD = 128
NH_LOC = 4          # heads per core
D_LOC = NH_LOC * HD  # 512
LAYER_INDEX = 3
SCALE = 1.0 / (math.sqrt(HD) * (LAYER_INDEX + 1))
N_CORES = 8
SB = 512            # S-block (matmul moving free dim)
HC = H // 128       # contraction chunks

# head-dim permutation: RoPE pair (x1_j, x2_j) -> rows (qd*32 + j%16,
# qd*32 + 16 + j%16) with qd = j//16, so the swap is within-quadrant.
_P_NEW2OLD = np.zeros(HD, dtype=np.int64)
_J_OF_P = np.zeros(HD, dtype=np.int64)
_SIGN_OF_P = np.zeros(HD, dtype=np.float32)
for _p in range(HD):
    _qd, _r = _p // 32, _p % 32
    _j = _qd * 16 + (_r % 16)
    _P_NEW2OLD[_p] = 2 * _j + (1 if _r >= 16 else 0)
    _J_OF_P[_p] = _j
    _SIGN_OF_P[_p] = 1.0 if _r >= 16 else -1.0
_SHUF_MASK = [(i + 16) % 32 for i in range(32)]

_BF16 = ml_dtypes.bfloat16
_NC_CACHE = {}


def _build_nc():
    import concourse.bacc as bacc
    import concourse.mybir as mybir
    import concourse.tile as tile

    f32 = mybir.dt.float32
    bf16 = mybir.dt.bfloat16
    Exp = mybir.ActivationFunctionType.Exp

    nc = bacc.Bacc("TRN2", target_bir_lowering=False, debug=False)

    xt_d = nc.dram_tensor("xt", [H, S], bf16, kind="ExternalInput")
    wqt_d = nc.dram_tensor("wqt", [H, D_LOC], bf16, kind="ExternalInput")
    wkt_d = nc.dram_tensor("wkt", [H, D_LOC], bf16, kind="ExternalInput")
    wvt_d = nc.dram_tensor("wvt", [H, D_LOC], bf16, kind="ExternalInput")
    wot_d = nc.dram_tensor("wot", [D_LOC, H], bf16, kind="ExternalInput")
    cos_d = nc.dram_tensor("cos_bc", [128, S], f32, kind="ExternalInput")
    sin_d = nc.dram_tensor("sin_pm", [128, S], f32, kind="ExternalInput")
    masks_d = nc.dram_tensor("masks", [4, 128, SB], bf16, kind="ExternalInput")
    ones_d = nc.dram_tensor("ones", [128, 128], bf16, kind="ExternalInput")
    out_d = nc.dram_tensor("out_partial", [S, H], f32, kind="ExternalOutput")

    n_sb = S // SB       # 4
    n_st = S // 128      # 16

    with tile.TileContext(nc) as tc:
        with (
            tc.tile_pool(name="const", bufs=1) as const_pool,
            tc.tile_pool(name="qkv", bufs=1) as qkv_pool,
        ):
            cos_t = const_pool.tile([128, S], f32, tag="cos")
            sin_t = const_pool.tile([128, S], f32, tag="sin")
            mask_t = const_pool.tile([128, 4, SB], bf16, tag="mask")
            ones_t = const_pool.tile([128, 128], bf16, tag="ones")
            nc.sync.dma_start(cos_t[:], cos_d[:, :])
            nc.sync.dma_start(sin_t[:], sin_d[:, :])
            nc.sync.dma_start(mask_t[:], masks_d[:, :, :].rearrange("j p f -> p j f"))
            nc.sync.dma_start(ones_t[:], ones_d[:, :])

            qT = qkv_pool.tile([128, NH_LOC, S], bf16, tag="qT")
            kT = qkv_pool.tile([128, NH_LOC, S], bf16, tag="kT")
            v_t = qkv_pool.tile([128, n_st, D_LOC], bf16, tag="v")

            # ---------------- Phase A: projections + RoPE ----------------
            with (
                tc.tile_pool(name="w", bufs=1) as w_pool,
                tc.tile_pool(name="xtp", bufs=2) as xt_pool,
                tc.tile_pool(name="rope", bufs=2) as rope_pool,
                tc.tile_pool(name="psA", bufs=2, space="PSUM") as psA,
            ):
                wq_t = w_pool.tile([128, HC, D_LOC], bf16, tag="wq")
                wk_t = w_pool.tile([128, HC, D_LOC], bf16, tag="wk")
                wv_t = w_pool.tile([128, HC, D_LOC], bf16, tag="wv")
                nc.sync.dma_start(
                    wq_t[:], wqt_d[:, :].rearrange("(hc p) d -> p hc d", p=128))
                nc.sync.dma_start(
                    wk_t[:], wkt_d[:, :].rearrange("(hc p) d -> p hc d", p=128))
                nc.sync.dma_start(
                    wv_t[:], wvt_d[:, :].rearrange("(hc p) d -> p hc d", p=128))

                xt_view = xt_d[:, :].rearrange(
                    "(hc p) (sb f) -> sb p hc f", p=128, f=SB)

                for sb in range(n_sb):
                    ssl = slice(sb * SB, (sb + 1) * SB)
                    xt_blk = xt_pool.tile([128, HC, SB], bf16, tag="xt")
                    nc.sync.dma_start(xt_blk[:], xt_view[sb])

                    for w_tile, dst, ptag in ((wq_t, qT, "pq"), (wk_t, kT, "pk")):
                        for h in range(NH_LOC):
                            hs = slice(h * 128, (h + 1) * 128)
                            ps = psA.tile([128, SB], f32, tag=ptag)
                            for hc in range(HC):
                                nc.tensor.matmul(
                                    ps[:], w_tile[:, hc, hs], xt_blk[:, hc, :],
                                    start=(hc == 0), stop=(hc == HC - 1))
                            # RoPE: dst = ps*cos + shuffle(ps)*sin_pm
                            t_sw = rope_pool.tile([128, SB], f32, tag="sw")
                            nc.vector.stream_shuffle(t_sw[:], ps[:], _SHUF_MASK)
                            t_pr = rope_pool.tile([128, SB], f32, tag="pr")
                            nc.vector.tensor_mul(t_pr[:], t_sw[:], sin_t[:, ssl])
                            t_cs = rope_pool.tile([128, SB], f32, tag="cs")
                            nc.vector.tensor_mul(t_cs[:], ps[:], cos_t[:, ssl])
                            nc.vector.tensor_add(dst[:, h, ssl], t_cs[:], t_pr[:])

                    for i in range(n_sb):
                        st = sb * 4 + i
                        isl = slice(i * 128, (i + 1) * 128)
                        ps = psA.tile([128, D_LOC], f32, tag="pv")
                        for hc in range(HC):
                            nc.tensor.matmul(
                                ps[:], xt_blk[:, hc, isl], wv_t[:, hc, :],
                                start=(hc == 0), stop=(hc == HC - 1))
                        nc.scalar.copy(v_t[:, st, :], ps[:])

            # ------------- Phase B: attention, Phase C: out proj -------------
            with (
                tc.tile_pool(name="wo", bufs=1) as wo_pool,
                tc.tile_pool(name="ot", bufs=1) as ot_pool,
            ):
                wo_t = wo_pool.tile([128, NH_LOC, H], bf16, tag="wo")
                nc.sync.dma_start(
                    wo_t[:], wot_d[:, :].rearrange("(dc p) o -> p dc o", p=128))
                ot_t = ot_pool.tile([128, NH_LOC, S], bf16, tag="ot")

                with (
                    tc.tile_pool(name="pt", bufs=8) as pt_pool,
                    tc.tile_pool(name="rcp", bufs=2) as rcp_pool,
                    tc.tile_pool(name="psB", bufs=1, space="PSUM") as psB,
                ):
                    for h in range(NH_LOC):
                        hs = slice(h * 128, (h + 1) * 128)
                        for qb in range(n_sb):
                            qsl = slice(qb * SB, (qb + 1) * SB)
                            nkt = 4 * (qb + 1)
                            ps_sum = psB.tile([128, SB], f32, tag="sum", bufs=2)
                            ps_o = psB.tile([128, SB], f32, tag="o", bufs=2)
                            # chunks of 4 k-tiles: scores+exp, sums, PV
                            for c0 in range(0, nkt, 4):
                                pts = []
                                for kt in range(c0, c0 + 4):
                                    ksl = slice(kt * 128, (kt + 1) * 128)
                                    ps_s = psB.tile(
                                        [128, SB], f32, tag="s", bufs=3)
                                    nc.tensor.matmul(
                                        ps_s[:], kT[:, h, ksl], qT[:, h, qsl],
                                        start=True, stop=True)
                                    pt = pt_pool.tile([128, SB], bf16, tag="pt")
                                    nc.scalar.activation(pt[:], ps_s[:], Exp)
                                    if kt >= 4 * qb:
                                        nc.vector.tensor_mul(
                                            pt[:], pt[:],
                                            mask_t[:, kt - 4 * qb, :])
                                    pts.append(pt)
                                for j, kt in enumerate(range(c0, c0 + 4)):
                                    nc.tensor.matmul(
                                        ps_sum[:], ones_t[:], pts[j][:],
                                        start=(kt == 0), stop=(kt == nkt - 1))
                                for j, kt in enumerate(range(c0, c0 + 4)):
                                    nc.tensor.matmul(
                                        ps_o[:], v_t[:, kt, hs], pts[j][:],
                                        start=(kt == 0), stop=(kt == nkt - 1))
                            rcp = rcp_pool.tile([128, SB], f32, tag="rcp")
                            nc.vector.reciprocal(rcp[:], ps_sum[:])
                            nc.vector.tensor_mul(
                                ot_t[:, h, qsl], ps_o[:], rcp[:])

                with (
                    tc.tile_pool(name="stage", bufs=6) as stage_pool,
                    tc.tile_pool(name="psC", bufs=6, space="PSUM") as psC,
                ):
                    for st in range(n_st):
                        stsl = slice(st * 128, (st + 1) * 128)
                        ps_outs = [psC.tile([128, SB], f32, tag="pc",
                                            name=f"pc_{st}_{hb}")
                                   for hb in range(4)]
                        for dc in range(NH_LOC):
                            for hb in range(4):
                                nc.tensor.matmul(
                                    ps_outs[hb][:],
                                    ot_t[:, dc, stsl],
                                    wo_t[:, dc, hb * SB:(hb + 1) * SB],
                                    start=(dc == 0), stop=(dc == NH_LOC - 1))
                        for hb in range(4):
                            o_sb = stage_pool.tile([128, SB], f32, tag="st")
                            nc.vector.tensor_copy(o_sb[:], ps_outs[hb][:])
                            nc.sync.dma_start(
                                out_d[stsl, hb * SB:(hb + 1) * SB], o_sb[:])

    nc.compile()
    return nc


def _host_prep(x, freq_cos, freq_sin, wq, wk, wv, wo):
    """Build the 8 per-core input maps."""
    cos_bc = np.ascontiguousarray(freq_cos.T[_J_OF_P, :]).astype(np.float32)
    sin_pm = np.ascontiguousarray(
        freq_sin.T[_J_OF_P, :] * _SIGN_OF_P[:, None]).astype(np.float32)

    f = np.arange(SB)[None, :]
    p = np.arange(128)[:, None]
    masks = np.stack(
        [(f - 128 * j - p >= 0) for j in range(4)]).astype(_BF16)
    ones = np.ones((128, 128), dtype=_BF16)

    xt_b = [np.ascontiguousarray(x[b].T).astype(_BF16) for b in range(2)]

    in_maps = []
    for c in range(N_CORES):
        b, g = c // 4, c % 4
        rows = slice(g * D_LOC, (g + 1) * D_LOC)
        wq_g = wq[rows, :].reshape(NH_LOC, HD, H)[:, _P_NEW2OLD, :]
        wk_g = wk[rows, :].reshape(NH_LOC, HD, H)[:, _P_NEW2OLD, :]
        in_maps.append({
            "xt": xt_b[b],
            "wqt": np.ascontiguousarray(
                wq_g.reshape(D_LOC, H).T * SCALE).astype(_BF16),
            "wkt": np.ascontiguousarray(
                wk_g.reshape(D_LOC, H).T).astype(_BF16),
            "wvt": np.ascontiguousarray(wv[rows, :].T).astype(_BF16),
            "wot": np.ascontiguousarray(wo[:, rows].T).astype(_BF16),
            "cos_bc": cos_bc,
            "sin_pm": sin_pm,
            "masks": masks,
            "ones": ones,
        })
    return in_maps


def _kernel_np_fallback(x, freq_cos, freq_sin, attention_mask, wq, wk, wv, wo):
    """Numpy fallback (only used if attention_mask is nonzero)."""
    B = x.shape[0]
    hd = H // NHEADS
    q = (x @ wq.T).reshape(B, S, NHEADS, hd)
    k = (x @ wk.T).reshape(B, S, NHEADS, hd)
    v = (x @ wv.T).reshape(B, S, NHEADS, hd)

    def rope(t):
        x1, x2 = t[..., ::2], t[..., 1::2]
        c = freq_cos[None, :, None, :]
        s = freq_sin[None, :, None, :]
        o = np.empty_like(t)
        o[..., ::2] = x1 * c - x2 * s
        o[..., 1::2] = x1 * s + x2 * c
        return o

    q, k = rope(q), rope(k)
    q = q.transpose(0, 2, 1, 3)
    k = k.transpose(0, 2, 1, 3)
    v = v.transpose(0, 2, 1, 3)
    att = np.einsum("bhqd,bhkd->bhqk", q, k) / np.sqrt(hd) / (LAYER_INDEX + 1)
    att = att + attention_mask
    causal = np.triu(np.full((S, S), -1e30, dtype=att.dtype), k=1)
    att = att + causal[None, None]
    att = att - att.max(axis=-1, keepdims=True)
    att = np.exp(att)
    att = att / att.sum(axis=-1, keepdims=True)
    out = np.einsum("bhqk,bhkd->bhqd", att, v)
    out = out.transpose(0, 2, 1, 3).reshape(B, S, H)
    return (out @ wo.T).astype(np.float32)


def kernel(x, freq_cos, freq_sin, attention_mask, wq, wk, wv, wo, **extra):
    x = np.asarray(x, dtype=np.float32)
    freq_cos = np.asarray(freq_cos, dtype=np.float32)
    freq_sin = np.asarray(freq_sin, dtype=np.float32)
    attention_mask = np.asarray(attention_mask, dtype=np.float32)
    wq = np.asarray(wq, dtype=np.float32)
    wk = np.asarray(wk, dtype=np.float32)
    wv = np.asarray(wv, dtype=np.float32)
    wo = np.asarray(wo, dtype=np.float32)

    if attention_mask.any():
        # the device kernel folds the (all-zero) pad mask away
        return _kernel_np_fallback(
            x, freq_cos, freq_sin, attention_mask, wq, wk, wv, wo)

    from concourse.bass_utils import run_bass_kernel_spmd

    if "nc" not in _NC_CACHE:
        _NC_CACHE["nc"] = _build_nc()
    nc = _NC_CACHE["nc"]

    in_maps = _host_prep(x, freq_cos, freq_sin, wq, wk, wv, wo)
    res = run_bass_kernel_spmd(nc, in_maps, list(range(N_CORES)))

    out = np.zeros((2, S, H), dtype=np.float32)
    for c in range(N_CORES):
        out[c // 4] += res.results[c]["out_partial"]
    return out
